# revision 15
# baseline (speedup 1.0000x reference)
"""Trainium2 Bass kernel for nn_AMM_module_55027120996423.

Computation: 3->1 channel 3x3 'same' conv + bias; softmax over the single
channel == 1.0; output hi = where(conv(x,w) + b + 0.5 < 0, 0, 1) as f32.

Strategy: pure data parallel over batch (32 images -> 4 per core x 8
cores), no collectives.  The host pre-packs x (fp16 cast, zero halos
baked in, partition-major per core) and the tiny banded weight matrix;
the device output is uint8 0/1 which the host expands to float32.

Compute mapping: output rows in 32-row col-tiles; a 32-row tile needs
34 input rows x 3 channels = 102 partitions, so channel/vertical taps
reduce in one pass and only the 3 horizontal taps need accumulating
matmuls.  Four col-tiles occupy the four 32-column PE groups and run
concurrently, so a 128-row group costs ~3 batch slots of N=512.  The
threshold -(0.5 + b) is baked into the DVE compare as a trace-time
immediate (nc is rebuilt if b changes between calls), so psum holds the
pure convolution and no bias/threshold tensors are loaded.

DMA plan: input loads are column-slices of ONE [102, 16*2056] fp16 SBUF
tile, chunked [1,2,2,...,2,1] groups and alternated across the two
HWDGE rings (sync/scalar) so each SDMA engine always has a second
queue to hide per-queue refill gaps; 8KB partition lines run at full
per-engine line rate.  The 19KB band rides first on the sync ring
(512B lines).  Stores ride gpsimd SWDGE (4KB concat packets) in
[4,4,4,2,1,1]-group chunks so the tail store is small.  8 warm-up
matmuls cover the PE HAM activity window until the first chunk lands.
"""

import os
from contextlib import ExitStack

import numpy as np

import concourse.tile as tile
from concourse import bacc, mybir
from concourse.bass_utils import run_bass_kernel_spmd

F32 = mybir.dt.float32
F16 = mybir.dt.float16
U8 = mybir.dt.uint8

B, C, H, W = 32, 3, 512, 512
NCORES = 8
BPC = B // NCORES          # images per core
TM = 32                    # output rows per col-tile
NJ = 4                     # col-tiles per group (4 x 32 = 128 rows)
GR = NJ * TM               # 128 output rows per group
NG = BPC * (H // GR)       # 16 groups per core
KP = C * (TM + 2)          # 102 matmul partitions
KPAD = 112                 # DMA partition count: the DMA splitter uses
                           # largest-divisor-of-P (<=16) many engines, so P
                           # must be 16-divisible to engage all 16 SDMA
                           # engines (102 -> 6 engines, 104 -> 13!)
CW = W + 2                 # padded tile width
GW = NJ * CW               # packed group width (4 col-tiles side by side)
LD_CHUNKS = [1, 2, 4, 4, 2, 2, 1]        # groups per load chunk
ST_CHUNKS = [4, 4, 4, 2, 1, 1]           # groups per store chunk
NWARM = 24                 # N=512 warm-up matmuls (bridge to first chunk)

LAST_EXEC_NS = None
LAST_RESULTS = None

_cache = {}


def _build_nc(thr: float):
    nc = bacc.Bacc("TRN2", target_bir_lowering=False, debug=False,
                   num_devices=NCORES)
    xp = nc.dram_tensor("xall", [KPAD, NG * GW], F16, kind="ExternalInput").ap()
    bandp = nc.dram_tensor("band", [KPAD, 256], F16, kind="ExternalInput").ap()
    ymp = nc.dram_tensor("ym", [GR, NG * W], U8, kind="ExternalOutput").ap()

    with tile.TileContext(nc) as tc, ExitStack() as ctx:
        const_pool = ctx.enter_context(tc.tile_pool(name="const", bufs=1))
        x_pool = ctx.enter_context(tc.tile_pool(name="xs", bufs=1))
        out_pool = ctx.enter_context(tc.tile_pool(name="outp", bufs=1))
        psum_pool = ctx.enter_context(tc.tile_pool(name="ps", bufs=6,
                                                   space="PSUM"))
        wps_pool = ctx.enter_context(tc.tile_pool(name="wps", bufs=1,
                                                  space="PSUM"))

        # band first on the sync ring: 512B lines, lands well before C0
        band_sb = const_pool.tile([KPAD, 256], F16)
        nc.sync.dma_start(band_sb[:], bandp)

        # chunked input loads: each chunk split across both HWDGE rings
        # (partitions 0:64 on sync, 64:112 on scalar, both 16-divisible)
        # so every SDMA engine always has a second queue to drain
        xt = x_pool.tile([KPAD, NG * GW], F16)
        off = 0
        for csz in LD_CHUNKS:
            c0, c1 = off * GW, (off + csz) * GW
            nc.sync.dma_start(xt[0:64, c0:c1], xp[0:64, c0:c1])
            nc.scalar.dma_start(xt[64:KPAD, c0:c1], xp[64:KPAD, c0:c1])
            off += csz

        # PE pre-warm: dependency-free matmuls cover the HAM activity
        # window until the first chunk lands (~4us).
        wsrc = const_pool.tile([KPAD, 512], F16)
        nc.vector.memset(wsrc[:], 0.0)
        wps = wps_pool.tile([126, 512], F32)
        for _ in range(NWARM):
            nc.tensor.matmul(wps[:], wsrc[:, 0:126], wsrc[:],
                             start=True, stop=True)

        ot = out_pool.tile([GR, NG * W], U8)
        off = 0
        for csz in ST_CHUNKS:
            for g in range(off, off + csz):
                pt = psum_pool.tile([GR, W], F32, tag="pt")
                for kx in range(3):
                    for j in range(NJ):
                        c0 = g * GW + j * CW + kx
                        nc.tensor.matmul(
                            pt[j * TM:(j + 1) * TM, :],
                            band_sb[0:KP, kx * TM:(kx + 1) * TM],
                            xt[0:KP, c0:c0 + W],
                            start=(kx == 0), stop=(kx == 2),
                            tile_position=(0, j * TM),
                        )
                nc.vector.tensor_scalar(out=ot[:, g * W:(g + 1) * W],
                                        in0=pt[:],
                                        scalar1=float(thr),
                                        scalar2=None,
                                        op0=mybir.AluOpType.is_ge)
            nc.gpsimd.dma_start(ymp[:, off * W:(off + csz) * W],
                                ot[:, off * W:(off + csz) * W])
            off += csz

    nc.compile()
    return nc


def _pack_inputs(x: np.ndarray, w: np.ndarray):
    """Host-side staging: fp16 cast + partition-major group packing."""
    x16 = x.astype(np.float16)
    # xpad[i, c, r+1, q+1] = x[i, c, r, q]; zero halos all around
    xpad = np.zeros((B, C, H + 2, CW), dtype=np.float16)
    xpad[:, :, 1:H + 1, 1:W + 1] = x16

    # group (img, t): col-tile j partition c*34+rl holds xpad row
    # 128t + 32j + rl (= x row 128t + 32j + rl - 1)
    NT = H // GR
    xg = np.zeros((B, NT, KPAD, NJ, CW), dtype=np.float16)
    for t in range(NT):
        for j in range(NJ):
            r0 = GR * t + TM * j
            sl = xpad[:, :, r0:r0 + TM + 2, :]        # [B, C, 34, CW]
            xg[:, t, :KP, j, :] = sl.reshape(B, KP, CW)
    xg = xg.reshape(B, NT, KPAD, NJ * CW)

    # per-core partition-major: [KP, NG*GW] with group g = img*NT + t
    xall = np.empty((NCORES, KPAD, NG * GW), dtype=np.float16)
    for i in range(NCORES):
        arr = xg[i * BPC:(i + 1) * BPC].reshape(NG, KPAD, GW)
        xall[i] = arr.transpose(1, 0, 2).reshape(KPAD, NG * GW)

    w16 = w.astype(np.float16)  # [1, C, 3, 3]
    band = np.zeros((KPAD, 256), dtype=np.float16)
    m = np.arange(TM)
    for c in range(C):
        for kx in range(3):
            for ky in range(3):
                band[c * (TM + 2) + m + ky, kx * TM + m] = w16[0, c, ky, kx]
    return xall, band


def kernel(x: np.ndarray, w: np.ndarray, b: np.ndarray) -> np.ndarray:
    global LAST_EXEC_NS, LAST_RESULTS
    x = np.ascontiguousarray(x, dtype=np.float32)
    w = np.ascontiguousarray(w, dtype=np.float32)
    b = np.ascontiguousarray(b, dtype=np.float32)

    # conv + b + 0.5 >= 0  <=>  conv >= thr
    thr = -(0.5 + float(b[0]))
    key = ("nc", thr)
    if key not in _cache:
        _cache.clear()
        _cache[key] = _build_nc(thr)
    nc = _cache[key]

    xall, band = _pack_inputs(x, w)
    in_maps = [{"xall": xall[i], "band": band} for i in range(NCORES)]

    kwargs = {}
    if os.environ.get("BASS_CONV_TRACE", "") not in ("", "0"):
        try:
            import ntff_shim
            ntff_shim.install()
            kwargs["trace"] = True
            kwargs["tmpdir"] = ntff_shim.new_trace_dir()
        except Exception:
            pass

    res = None
    for attempt in range(3):
        try:
            res = run_bass_kernel_spmd(nc, in_maps,
                                       core_ids=list(range(NCORES)), **kwargs)
            break
        except Exception:
            if attempt == 2:
                raise
    LAST_EXEC_NS = res.exec_time_ns
    LAST_RESULTS = res

    NT = H // GR
    out = np.empty((B, 1, H, W), dtype=np.float32)
    for i in range(NCORES):
        ym = res.results[i]["ym"]  # [128, NG*512] u8
        grp = ym.reshape(GR, NG, W).transpose(1, 0, 2)  # [NG, 128, 512]
        full = grp.reshape(BPC, NT * GR, W)             # [img, 512, 512]
        out[i * BPC:(i + 1) * BPC, 0] = (full != 0)
    return out


# revision 21
# speedup vs baseline: 1.1274x; 1.1274x over previous
"""Trainium2 Bass kernel for nn_AMM_module_55027120996423.

Computation: 3->1 channel 3x3 'same' conv + bias; softmax over the single
channel == 1.0; output hi = where(conv(x,w) + b + 0.5 < 0, 0, 1) as f32.

Strategy: pure data parallel over batch (32 images -> 4 per core x 8
cores), no collectives.  The host pre-packs x (fp16 cast, zero halos
baked in, partition-major per core) and the tiny banded weight matrix;
the device output is uint8 0/1 which the host expands to float32.

Compute mapping: output rows in 32-row col-tiles; a 32-row tile needs
34 input rows x 3 channels = 102 partitions, so channel/vertical taps
reduce in one pass and only the 3 horizontal taps need accumulating
matmuls.  Four col-tiles occupy the four 32-column PE groups and run
concurrently, so a 128-row group costs ~3 batch slots of N=512.  The
threshold -(0.5 + b) is baked into the DVE compare as a trace-time
immediate (nc is rebuilt if b changes between calls).

RAW bass (no TileContext): the Tile scheduler's epilogue (sem reset
ladder + barrier butterfly) costs ~9us on a ~35us kernel, so the sync
protocol is hand-rolled with 4 semaphores:
  load_sem  : each input DMA (band + 7 chunks, all on the sync HWDGE
              ring, strict FIFO) then_inc(16); matmuls wait cumulative.
  mm_sem    : last matmul of each 128-row group then_inc(1) (matmuls
              complete in pc order); the DVE threshold waits g+1.
  thr_sem   : each threshold then_inc(1); group g's first matmul waits
              g-5 before reusing psum bank g%6; stores wait chunk counts.
  store_sem : SWDGE stores then_inc(16); final wait + sem clears.
Input loads are column-slices of ONE [112, 16*2056] fp16 SBUF tile
(112 partitions: the DMA splitter engages largest-divisor-of-P<=16
engines, so P must be 16-divisible).  24 warm-up matmuls keep the PE
HAM activity window busy until the first chunk lands.
"""

import os
from contextlib import ExitStack

import numpy as np

from concourse import bacc, mybir
from concourse.bass_utils import run_bass_kernel_spmd

F32 = mybir.dt.float32
F16 = mybir.dt.float16
U8 = mybir.dt.uint8

B, C, H, W = 32, 3, 512, 512
NCORES = 8
BPC = B // NCORES          # images per core
TM = 32                    # output rows per col-tile
NJ = 4                     # col-tiles per group (4 x 32 = 128 rows)
GR = NJ * TM               # 128 output rows per group
NG = BPC * (H // GR)       # 16 groups per core
KP = C * (TM + 2)          # 102 matmul partitions
KPAD = 112                 # DMA partition count (16-divisible, see above)
CW = W + 2                 # padded tile width
GW = NJ * CW               # packed group width (4 col-tiles side by side)
LD_CHUNKS = [1, 2, 4, 4, 2, 2, 1]        # groups per load chunk
ST_CHUNKS = [4, 4, 4, 2, 1, 1]           # groups per store chunk
NPSUM = 6                  # psum banks cycled by groups
NWARM = 16                 # N=512 warm-up matmuls (bridge to first chunk)

LAST_EXEC_NS = None
LAST_RESULTS = None

_cache = {}


def _build_nc(thr: float):
    nc = bacc.Bacc("TRN2", target_bir_lowering=False, debug=False,
                   num_devices=NCORES)
    xp = nc.dram_tensor("xall", [KPAD, NG * GW], F16, kind="ExternalInput").ap()
    bandp = nc.dram_tensor("band", [KPAD, 256], F16, kind="ExternalInput").ap()
    ymp = nc.dram_tensor("ym", [GR, NG * W], U8, kind="ExternalOutput").ap()

    # group g lives in load chunk chunk_of[g]
    chunk_of = []
    for ci, csz in enumerate(LD_CHUNKS):
        chunk_of += [ci] * csz

    with ExitStack() as ctx:
        band_sem = ctx.enter_context(nc.semaphore("band_sem"))
        ld_sems = [ctx.enter_context(nc.semaphore(f"ld_sem{c}"))
                   for c in range(len(LD_CHUNKS))]
        mm_sem = ctx.enter_context(nc.semaphore("mm_sem"))
        thr_sem = ctx.enter_context(nc.semaphore("thr_sem"))
        store_sem = ctx.enter_context(nc.semaphore("store_sem"))

        band_sb = ctx.enter_context(
            nc.sbuf_tensor("band_sb", [KPAD, 256], F16))
        xt = ctx.enter_context(
            nc.sbuf_tensor("xt", [KPAD, NG * GW], F16))
        ot = ctx.enter_context(
            nc.sbuf_tensor("ot", [GR, NG * W], U8))
        wsrc = ctx.enter_context(
            nc.sbuf_tensor("wsrc", [KPAD, 512], F16))
        pts = [ctx.enter_context(
            nc.psum_tensor(f"pt{i}", [GR, W], F32)) for i in range(NPSUM)]
        wps = ctx.enter_context(nc.psum_tensor("wps", [126, 512], F32))

        # ---- sync engine: band first, then input chunks, strict FIFO.
        # One sem per DMA: a single cumulative sem is unsound because the
        # 16 SDMA engines skew across chunks (chunk c+1 increments can
        # stand in for a straggler engine's chunk-c lines).
        nc.sync.dma_start(band_sb[:, :], bandp).then_inc(band_sem, 16)
        off = 0
        for c, csz in enumerate(LD_CHUNKS):
            nc.sync.dma_start(xt[:, off * GW:(off + csz) * GW],
                              xp[:, off * GW:(off + csz) * GW]
                              ).then_inc(ld_sems[c], 16)
            off += csz

        # ---- vector engine: zero the warm-up source, then thresholds ----
        nc.vector.memset(wsrc[:, :], 0.0)

        # ---- tensor engine: warm-ups, then 12 matmuls per group ----
        for _ in range(NWARM):
            nc.tensor.matmul(wps[:, :], wsrc[:, 0:126], wsrc[:, :],
                             start=True, stop=True)
        nc.tensor.wait_ge(band_sem, 16)
        for g in range(NG):
            pt = pts[g % NPSUM]
            if g == 0 or chunk_of[g] != chunk_of[g - 1]:
                nc.tensor.wait_ge(ld_sems[chunk_of[g]], 16)
            if g >= NPSUM:
                nc.tensor.wait_ge(thr_sem, g - (NPSUM - 1))
            inst = None
            for kx in range(3):
                for j in range(NJ):
                    c0 = g * GW + j * CW + kx
                    inst = nc.tensor.matmul(
                        pt[j * TM:(j + 1) * TM, :],
                        band_sb[0:KP, kx * TM:(kx + 1) * TM],
                        xt[0:KP, c0:c0 + W],
                        start=(kx == 0), stop=(kx == 2),
                        tile_position=(0, j * TM),
                    )
            inst.then_inc(mm_sem, 1)

        # ---- vector engine: per-group threshold psum -> u8 ----
        for g in range(NG):
            nc.vector.wait_ge(mm_sem, g + 1)
            nc.vector.tensor_scalar(out=ot[:, g * W:(g + 1) * W],
                                    in0=pts[g % NPSUM][:, :],
                                    scalar1=float(thr),
                                    scalar2=None,
                                    op0=mybir.AluOpType.is_ge
                                    ).then_inc(thr_sem, 1)

        # ---- chunked stores: SWDGE (4KB concat packets) except the last,
        # which rides the scalar HWDGE ring for its lower latency ----
        off = 0
        for si, csz in enumerate(ST_CHUNKS):
            last = si == len(ST_CHUNKS) - 1
            eng = nc.scalar if last else nc.gpsimd
            eng.wait_ge(thr_sem, off + csz)
            eng.dma_start(ymp[:, off * W:(off + csz) * W],
                          ot[:, off * W:(off + csz) * W]
                          ).then_inc(store_sem, 16)
            off += csz

        # ---- epilogue: wait for stores, zero the sems for re-execution ----
        nc.gpsimd.wait_ge(store_sem, 16 * len(ST_CHUNKS))
        for sem in [band_sem, mm_sem, thr_sem, store_sem] + ld_sems:
            nc.gpsimd.sem_clear(sem)

    nc.compile()
    return nc


def _pack_inputs(x: np.ndarray, w: np.ndarray):
    """Host-side staging: fp16 cast + partition-major group packing."""
    x16 = x.astype(np.float16)
    # xpad[i, c, r+1, q+1] = x[i, c, r, q]; zero halos all around
    xpad = np.zeros((B, C, H + 2, CW), dtype=np.float16)
    xpad[:, :, 1:H + 1, 1:W + 1] = x16

    # group (img, t): col-tile j partition c*34+rl holds xpad row
    # 128t + 32j + rl (= x row 128t + 32j + rl - 1)
    NT = H // GR
    xg = np.zeros((B, NT, KPAD, NJ, CW), dtype=np.float16)
    for t in range(NT):
        for j in range(NJ):
            r0 = GR * t + TM * j
            sl = xpad[:, :, r0:r0 + TM + 2, :]        # [B, C, 34, CW]
            xg[:, t, :KP, j, :] = sl.reshape(B, KP, CW)
    xg = xg.reshape(B, NT, KPAD, NJ * CW)

    # per-core partition-major: [KPAD, NG*GW] with group g = img*NT + t
    xall = np.empty((NCORES, KPAD, NG * GW), dtype=np.float16)
    for i in range(NCORES):
        arr = xg[i * BPC:(i + 1) * BPC].reshape(NG, KPAD, GW)
        xall[i] = arr.transpose(1, 0, 2).reshape(KPAD, NG * GW)

    w16 = w.astype(np.float16)  # [1, C, 3, 3]
    band = np.zeros((KPAD, 256), dtype=np.float16)
    m = np.arange(TM)
    for c in range(C):
        for kx in range(3):
            for ky in range(3):
                band[c * (TM + 2) + m + ky, kx * TM + m] = w16[0, c, ky, kx]
    return xall, band


def kernel(x: np.ndarray, w: np.ndarray, b: np.ndarray) -> np.ndarray:
    global LAST_EXEC_NS, LAST_RESULTS
    x = np.ascontiguousarray(x, dtype=np.float32)
    w = np.ascontiguousarray(w, dtype=np.float32)
    b = np.ascontiguousarray(b, dtype=np.float32)

    # conv + b + 0.5 >= 0  <=>  conv >= thr
    thr = -(0.5 + float(b[0]))
    key = ("nc", thr)
    if key not in _cache:
        _cache.clear()
        _cache[key] = _build_nc(thr)
    nc = _cache[key]

    xall, band = _pack_inputs(x, w)
    in_maps = [{"xall": xall[i], "band": band} for i in range(NCORES)]

    kwargs = {}
    if os.environ.get("BASS_CONV_TRACE", "") not in ("", "0"):
        try:
            import ntff_shim
            ntff_shim.install()
            kwargs["trace"] = True
            kwargs["tmpdir"] = ntff_shim.new_trace_dir()
        except Exception:
            pass

    res = None
    for attempt in range(3):
        try:
            res = run_bass_kernel_spmd(nc, in_maps,
                                       core_ids=list(range(NCORES)), **kwargs)
            break
        except Exception:
            if attempt == 2:
                raise
    LAST_EXEC_NS = res.exec_time_ns
    LAST_RESULTS = res

    NT = H // GR
    out = np.empty((B, 1, H, W), dtype=np.float32)
    for i in range(NCORES):
        ym = res.results[i]["ym"]  # [128, NG*512] u8
        grp = ym.reshape(GR, NG, W).transpose(1, 0, 2)  # [NG, 128, 512]
        full = grp.reshape(BPC, NT * GR, W)             # [img, 512, 512]
        out[i * BPC:(i + 1) * BPC, 0] = (full != 0)
    return out
